# revision 61
# baseline (speedup 1.0000x reference)
"""Paged KV-cache decode attention with ALiBi (Baichuan-style), fused
QKV + attention + output projection, tensor-parallel over heads across
8 Trainium2 NeuronCores.

v6: fp16 everywhere + ALiBi window truncation with balanced head
permutation.

ALiBi slopes decay geometrically, so for most heads only the last
`win_h = ceil(40/slope_h)` positions can contribute: everything earlier
has additive bias < -40, i.e. softmax weight < e^-34 ~ 1e-15 of the
winner. We therefore (a) rank all 40 heads by window size, (b) give
each core one head from each octile (balanced shards: every core gets
the same per-slot chunk capacities, so one shared SPMD program works),
and (c) host-pack ONLY the needed trailing chunks of K^T/V per
(slot, seq). This cuts K/V HBM traffic AND the attention matmul count
by ~45% on every core.

Layout strategy (per core, 5 head-slots):
  - qT/kT computed as [640, 4] (head-dim on partitions) so scores matmuls
    need no transposes and the K-cache new-token scatter is a same-partition
    SBUF copy.
  - v computed as [4, 640] (natural) so the V new-token scatter is a tiny
    SBUF->SBUF DMA row write (scalar/HWDGE queue, grouped per head).
  - K packed host-side as [128(d), sum_chunks*128] (K^T), V as
    [128(t%128), sum_chunks, 128(d)]; ONE DMA per head-slot, single-use
    exact-size SBUF tiles (no pool-reuse waits in the DMA stream).
  - One explicitly-ordered bulk DMA stream on the gpsimd queue:
    weights first (the last v-weight chunk gates all attention), then
    K/V slots, then o_proj weights (consumed at DMA pace at the end).
  - Per-head attention emits all 4 seqs' score matmuls before the 4 AV
    chains so the exp round-trips hide behind other seqs' scores.
  - softmax without max-subtraction; masking via host-precomputed
    additive fp32 bias (-1e30).
  - o_proj in natural orientation (lhsT = tiny attn columns, ow rides
    the 512-wide moving side); host sums the 8 partial outputs.
"""

import math
import os
import sys
from contextlib import ExitStack

import numpy as np

sys.path.insert(0, "/opt/trn_rl_repo")

B = 4
E = 5120
H = 40
D = 128
BS = 16
NB = 512
MB = 128
S = MB * BS  # 2048
NCORES = 8
HPC = H // NCORES   # 5 head-slots per core
EPC = HPC * D       # 640

NEG = -1.0e30
GK = 10             # E-chunks (of 128) per qkv weight DMA group
TCUT = 26.0         # alibi bias cutoff: positions with bias < -TCUT dropped
                    # (dropped softmax weight <= ~e^-14 relative: negligible)


def _alibi_slopes(num_heads):
    cp2 = 2 ** int(math.floor(math.log2(num_heads)))
    base = 2.0 ** (-(2.0 ** (-(math.log2(cp2) - 3))))
    slopes = base ** np.arange(1, cp2 + 1, dtype=np.float64)
    if cp2 != num_heads:
        extra_base = 2.0 ** (-(2.0 ** (-(math.log2(2 * cp2) - 3))))
        n_rem = min(cp2, num_heads - cp2)
        extra = extra_base ** np.arange(1, 1 + 2 * n_rem, 2, dtype=np.float64)
        slopes = np.concatenate([slopes, extra])
    return slopes.astype(np.float32)


def _head_partition(pos, nch):
    """Rank heads by alibi window, assign core c slot s <- rank[s*8+c].
    Returns (order, m) where m[s][b] = kept trailing chunks for slot s."""
    win = np.ceil(TCUT / _alibi_slopes(H).astype(np.float64)).astype(np.int64)
    order = np.argsort(win, kind="stable")
    m = []
    for s in range(HPC):
        wmax = int(win[order[s * NCORES:(s + 1) * NCORES]].max())
        m.append(tuple(nch[b] - max(0, (pos[b] - wmax) // 128) for b in range(B)))
    return order, tuple(m)


_PROGRAM_CACHE = {}
LAST_RESULTS = None  # BassKernelResults of the most recent run (for test.py)


def _build_program(pos, nch, m):
    """Build the SPMD Bass program. pos/nch/m are baked statically (same
    for all cores; per-core data varies only via inputs)."""
    import concourse.bacc as bacc
    import concourse.bass as bass
    import concourse.tile as tile
    from concourse import mybir

    f32 = mybir.dt.float32
    f16 = mybir.dt.float16
    nc = bacc.Bacc()

    scnt = [sum(m[s]) for s in range(HPC)]   # chunks per slot
    soff = [0]
    for s in range(HPC):
        soff.append(soff[-1] + scnt[s])
    KCH = soff[-1]
    # chunk offset of (s, b) within slot s's tile
    moff = [[sum(m[s][:b]) for b in range(B)] for s in range(HPC)]
    c0 = [[nch[b] - m[s][b] for b in range(B)] for s in range(HPC)]

    hT = nc.declare_dram_parameter("hT", [128, 40 * B], f16, isOutput=False)
    qkvw = nc.declare_dram_parameter("qkvw", [3, 128, 40, EPC], f16, isOutput=False)
    ow = nc.declare_dram_parameter("ow", [128, HPC * E], f16, isOutput=False)
    kt = nc.declare_dram_parameter("kt", [128, KCH * 128], f16, isOutput=False)
    vt = nc.declare_dram_parameter("vt", [128, KCH, D], f16, isOutput=False)
    bias = nc.declare_dram_parameter("bias", [128, B * HPC * 16], f32, isOutput=False)
    outT = nc.declare_dram_parameter("outT", [B, E], f32, isOutput=True)

    NG = 40 // GK  # weight DMA groups per tensor

    with tile.TileContext(nc) as tc, ExitStack() as ctx:
        consts = ctx.enter_context(tc.tile_pool(name="consts", bufs=1))
        wpool = ctx.enter_context(tc.tile_pool(name="wpool", bufs=4))
        tmp = ctx.enter_context(tc.tile_pool(name="tmp", bufs=4))
        psum = ctx.enter_context(tc.tile_pool(name="psum", bufs=8, space="PSUM"))

        hT_sb = consts.tile([128, 40 * B], f16)          # (E%128, (Echunk, b))
        bias_sb = consts.tile([128, B * HPC * 16], f32)  # (t%128, (b, s, chunk))
        ow_sb = consts.tile([128, HPC * E], f16)
        qT_sb = consts.tile([128, HPC * B], f16)   # col = s*B + b ; partition = d
        kT_sb = consts.tile([128, HPC * B], f16)
        v_sb = consts.tile([B, EPC], f16)          # natural v rows
        colsum_sb = consts.tile([128, HPC * B], f32)
        aoT_sb = consts.tile([128, HPC * B], f32)  # unnormalized attn@V ^T
        out_sb = consts.tile([B, E], f32)          # natural o_proj output

        ones_col = consts.tile([128, 1], f32)
        nc.vector.memset(ones_col[:], 1.0)
        ones_row = consts.tile([1, 128], f32)
        nc.vector.memset(ones_row[:], 1.0)

        # per-slot single-use exact-size K/V tiles (no pool-reuse waits)
        Kts = [consts.tile([128, scnt[s] * 128], f16, name=f"K{s}") for s in range(HPC)]
        Vts = [consts.tile([128, scnt[s], D], f16, name=f"V{s}") for s in range(HPC)]

        # ---- the bulk DMA stream: ONE queue (gpsimd/SWDGE), explicitly
        # ordered. All weight groups land before the K/V slots (the last
        # v-weight chunk gates the whole attention phase); ow last so the
        # o_proj tail is DMA-paced.
        wq, wk, wv = [], [], []

        def qkv_group(w, lst):
            t = wpool.tile([128, GK, EPC], f16, tag="w", name=f"w{w}_{len(lst)}")
            nc.gpsimd.dma_start(
                out=t[:], in_=qkvw[w, :, len(lst) * GK:(len(lst) + 1) * GK, :]
            )
            lst.append(t)

        nc.gpsimd.dma_start(out=hT_sb[:], in_=hT[:])
        nc.gpsimd.dma_start(out=bias_sb[:], in_=bias[:])
        for g in range(NG):
            qkv_group(0, wq)
        nc.gpsimd.dma_start(out=Kts[0][:], in_=kt[:, soff[0] * 128: soff[1] * 128])
        for g in range(NG):
            qkv_group(1, wk)
        # all K tiles right after the k-weights (only ~3MB now): the DVE
        # K-scatters -- which Tile hoists ahead of the bias-adds -- unblock
        # as soon as kT is ready instead of gating on late K arrivals.
        for s in range(1, HPC):
            nc.gpsimd.dma_start(out=Kts[s][:], in_=kt[:, soff[s] * 128: soff[s + 1] * 128])
        for g in range(NG):
            qkv_group(2, wv)
        for s in range(HPC):
            nc.gpsimd.dma_start(out=Vts[s][:], in_=vt[:, soff[s]: soff[s + 1], :])
        # ow in jg-major chunks: o_proj group jg only needs chunk jg//2,
        # so the projection pipelines at DMA pace behind the stream tail.
        for oc in range(5):
            w = HPC * E // 5
            nc.gpsimd.dma_start(out=ow_sb[:, oc * w:(oc + 1) * w], in_=ow[:, oc * w:(oc + 1) * w])

        # ---- fused QKV projection ----
        # q,k transposed orientation: psum[oc] [128, B] accumulated over 40
        # E-chunks; lhsT = W chunk [128(E), 128(outcol)], rhs = hT chunk [128(E), B].
        for w, lst in ((0, wq), (1, wk)):  # 0=q (pre-scaled on host), 1=k
            dst = qT_sb if w == 0 else kT_sb
            ps = [psum.tile([128, B], f32, tag="ps", name=f"ps_qk{w}_{i}") for i in range(HPC)]
            for g in range(NG):
                wt = lst[g]
                for oc in range(HPC):
                    for kl in range(GK):
                        kc = g * GK + kl
                        nc.tensor.matmul(
                            ps[oc][:],
                            lhsT=wt[:, kl, oc * 128:(oc + 1) * 128],
                            rhs=hT_sb[:, kc * B:(kc + 1) * B],
                            start=(kc == 0),
                            stop=(kc == 39),
                        )
            for oc in range(HPC):
                nc.scalar.copy(dst[:, oc * B:(oc + 1) * B], ps[oc][:])

        # v natural orientation: psum [B, 640] (two banks: 512 + 128),
        # lhsT = hT chunk [128(E), B], rhs = Wv chunk [128(E), 640].
        v_ps0 = psum.tile([B, 512], f32, tag="ps")
        v_ps1 = psum.tile([B, EPC - 512], f32, tag="ps")
        for g in range(NG):
            wt = wv[g]
            for kl in range(GK):
                kc = g * GK + kl
                nc.tensor.matmul(
                    v_ps0[:],
                    lhsT=hT_sb[:, kc * B:(kc + 1) * B],
                    rhs=wt[:, kl, :512],
                    start=(kc == 0),
                    stop=(kc == 39),
                )
                nc.tensor.matmul(
                    v_ps1[:],
                    lhsT=hT_sb[:, kc * B:(kc + 1) * B],
                    rhs=wt[:, kl, 512:],
                    start=(kc == 0),
                    stop=(kc == 39),
                )
        nc.scalar.copy(v_sb[:, :512], v_ps0[:])
        nc.scalar.copy(v_sb[:, 512:], v_ps1[:])

        # ---- attention, head-slot-major; per head: scatters, then all 4
        # seqs' scores, then adds/exps, then the 4 AV chains, so the
        # DVE/ACT round-trips hide behind other seqs' score matmuls.
        for s in range(HPC):
            Kt = Kts[s]
            Vt = Vts[s]
            lpos = [moff[s][b] * 128 + (pos[b] // 128 - c0[s][b]) * 128 + pos[b] % 128
                    for b in range(B)]
            # V new-token scatter rows (cross-partition -> DMA). On the sync
            # queue, which is otherwise idle until the final store: the
            # issue waits (v_sb + Vt arrival) can't block exps or adds.
            for b in range(B):
                p = pos[b]
                nc.sync.dma_start(
                    out=Vt[p % 128: p % 128 + 1, moff[s][b] + p // 128 - c0[s][b], :],
                    in_=v_sb[b:b + 1, s * D:(s + 1) * D],
                )
            # K new-token scatter columns (same partitions: DVE copy)
            for b in range(B):
                nc.vector.tensor_copy(
                    Kt[:, lpos[b]: lpos[b] + 1], kT_sb[:, (s * B + b):(s * B + b) + 1]
                )
            sc_ps = [psum.tile([128, 16], f32, tag="ps", name=f"sc_{s}_{b}") for b in range(B)]
            for b in range(B):
                for c in range(m[s][b]):
                    nc.tensor.matmul(
                        sc_ps[b][:, c:c + 1],
                        lhsT=Kt[:, (moff[s][b] + c) * 128:(moff[s][b] + c + 1) * 128],
                        rhs=qT_sb[:, s * B + b: s * B + b + 1],
                        start=True,
                        stop=True,
                    )
            attn = []
            for b in range(B):
                n = m[s][b]
                col = s * B + b
                # deep rings (tiles are 32-64B/partition): a slot's exps must
                # never wait on the previous slot's AV chains to free buffers
                s_sb = tmp.tile([128, 16], f32, tag="s", name=f"s_{s}_{b}", bufs=8)
                nc.vector.tensor_add(
                    s_sb[:, :n],
                    sc_ps[b][:, :n],
                    bias_sb[:, (b * HPC + s) * 16:(b * HPC + s) * 16 + n],
                )
                attn_sb = tmp.tile([128, 16], f16, tag="attn", name=f"at_{s}_{b}",
                                   bufs=HPC * B)
                nc.scalar.activation(
                    attn_sb[:, :n],
                    s_sb[:, :n],
                    func=mybir.ActivationFunctionType.Exp,
                    accum_out=colsum_sb[:, col:col + 1],
                )
                attn.append(attn_sb)
            for b in range(B):
                n = m[s][b]
                col = s * B + b
                ao_ps = psum.tile([128, 1], f32, tag="ps", name=f"ao_{s}_{b}")
                for c in range(n):
                    nc.tensor.matmul(
                        ao_ps[:],
                        lhsT=Vt[:, moff[s][b] + c, :],
                        rhs=attn[b][:, c:c + 1],
                        start=(c == 0),
                        stop=(c == n - 1),
                    )
                nc.scalar.copy(aoT_sb[:, col:col + 1], ao_ps[:])

        # ---- softmax normalization (batched over all 20 (s,b)) ----
        sums_ps = psum.tile([1, HPC * B], f32, tag="ps")
        nc.tensor.matmul(
            sums_ps[:], lhsT=ones_col[:], rhs=colsum_sb[:], start=True, stop=True
        )
        recip_sb = tmp.tile([1, HPC * B], f32, tag="recip")
        nc.vector.reciprocal(recip_sb[:], sums_ps[:])
        rb_ps = psum.tile([128, HPC * B], f32, tag="ps")
        nc.tensor.matmul(
            rb_ps[:], lhsT=ones_row[:], rhs=recip_sb[:], start=True, stop=True
        )
        recip_b = tmp.tile([128, HPC * B], f32, tag="recipb")
        nc.vector.tensor_copy(recip_b[:], rb_ps[:])
        attn_nT = consts.tile([128, HPC * B], f16)
        nc.vector.tensor_mul(attn_nT[:], aoT_sb[:], recip_b[:])

        # ---- output projection (natural): out[b, j] ----
        # lhsT = attn_nT slice [128(hd), B] (4-col weight load, ~free);
        # rhs = ow chunk [128(hd%128), 512] moving at 1 col/cycle.
        for jg in range(E // 512):
            ops = psum.tile([B, 512], f32, tag="ps", name=f"ps_o{jg}")
            for hh in range(HPC):
                j0 = jg * (HPC * 512) + hh * 512
                nc.tensor.matmul(
                    ops[:],
                    lhsT=attn_nT[:, hh * B:(hh + 1) * B],
                    rhs=ow_sb[:, j0: j0 + 512],
                    start=(hh == 0),
                    stop=(hh == HPC - 1),
                )
            # alternate evacuation engines so the copy chain pipelines
            if jg % 2 == 0:
                nc.scalar.copy(out_sb[:, jg * 512:(jg + 1) * 512], ops[:])
            else:
                nc.vector.tensor_copy(out_sb[:, jg * 512:(jg + 1) * 512], ops[:])

        nc.sync.dma_start(out=outT[:, :E // 2], in_=out_sb[:, :E // 2])
        nc.sync.dma_start(out=outT[:, E // 2:], in_=out_sb[:, E // 2:])

    nc.compile()  # Bacc finalize: splits multi-waits (matmul 1-wait limit)
    return nc


def _prepare_core_inputs(core, hidden16, qkv16, o16, k16, v16, bt, sl, pos, nch,
                         order, m):
    """Per-core staged arrays with the window-permuted head assignment."""
    heads = [int(order[s * NCORES + core]) for s in range(HPC)]

    # partition-major: qkvw[w, p, kc, c] = W[w, kc*128 + p, head cols c]
    qkvw = np.ascontiguousarray(
        qkv16.reshape(3, E, H, D)[:, :, heads, :]
        .reshape(3, 40, 128, EPC).transpose(0, 2, 1, 3)
    )

    scnt = [sum(m[s]) for s in range(HPC)]
    KCH = sum(scnt)
    moff = [[sum(m[s][:b]) for b in range(B)] for s in range(HPC)]
    soff = [0]
    for s in range(HPC):
        soff.append(soff[-1] + scnt[s])
    c0 = [[nch[b] - m[s][b] for b in range(B)] for s in range(HPC)]

    kg = k16[:, heads]  # [NB, HPC, BS, D]
    vg = v16[:, heads]
    kt = np.zeros((D, KCH * 128), np.float16)
    vt = np.zeros((128, KCH, D), np.float16)
    for b in range(B):
        sd = nch[b] * 128
        blocks = bt[b][: sd // BS]
        kk = kg[blocks].transpose(1, 0, 2, 3).reshape(HPC, sd, D)
        vv = vg[blocks].transpose(1, 0, 2, 3).reshape(HPC, sd, D)
        for s in range(HPC):
            base = soff[s] + moff[s][b]
            n = m[s][b]
            ksl = kk[s, c0[s][b] * 128: sd]              # [n*128, D]
            kt[:, base * 128: (base + n) * 128] = ksl.T
            vt[:, base: base + n, :] = vv[s, c0[s][b] * 128: sd].reshape(
                n, 128, D).transpose(1, 0, 2)

    slopes = _alibi_slopes(H)[heads]
    t_in = np.arange(128)[:, None]
    biasa = np.full((128, B, HPC, 16), NEG, np.float32)
    for b in range(B):
        for s in range(HPC):
            n = m[s][b]
            tg = ((c0[s][b] + np.arange(n))[None, :] * 128 + t_in).astype(np.float32)
            val = slopes[s] * (tg - np.float32(pos[b]))
            val[tg >= sl[b]] = NEG
            biasa[:, b, s, :n] = val

    hTf = np.ascontiguousarray(
        hidden16.T.reshape(40, 128, B).transpose(1, 0, 2).reshape(128, 40 * B)
    )

    # ow pre-transposed, jg-major: owr[p, jg*HPC*512 + s*512 + j'] =
    # o_proj_weight[heads[s]*128 + p, jg*512 + j']
    owr = np.ascontiguousarray(
        o16.reshape(H, D, E)[heads].reshape(HPC, 128, E // 512, 512)
        .transpose(1, 2, 0, 3).reshape(128, HPC * E)
    )

    return dict(
        hT=hTf,
        qkvw=qkvw,
        ow=owr,
        kt=kt,
        vt=vt,
        bias=np.ascontiguousarray(biasa.reshape(128, B * HPC * 16)),
    )


def kernel(**inputs):
    global LAST_RESULTS
    hidden = np.asarray(inputs["hidden_states"], np.float32)
    qkv_w = np.asarray(inputs["qkv_weight"], np.float32)
    o_w = np.asarray(inputs["o_proj_weight"], np.float32)
    k_cache = np.asarray(inputs["k_cache"], np.float32)
    v_cache = np.asarray(inputs["v_cache"], np.float32)
    bt = np.asarray(inputs["block_tables"]).astype(np.int64)
    sl = np.asarray(inputs["sequence_lengths"]).astype(np.int64)

    pos = tuple(int(x) - 1 for x in sl)
    nch = tuple(int(math.ceil(int(x) / 128)) for x in sl)
    order, m = _head_partition(pos, nch)

    # cast once to fp16 (q pre-scaled by 1/sqrt(D) before the cast)
    hidden16 = hidden.astype(np.float16)
    qkv16 = qkv_w.copy()
    qkv16[0] *= np.float32(D ** -0.5)
    qkv16 = qkv16.astype(np.float16)
    o16 = o_w.astype(np.float16)
    k16 = k_cache.astype(np.float16)
    v16 = v_cache.astype(np.float16)

    in_maps = [
        _prepare_core_inputs(c, hidden16, qkv16, o16, k16, v16, bt, sl, pos, nch,
                             order, m)
        for c in range(NCORES)
    ]

    key = (pos, nch, m)
    if key not in _PROGRAM_CACHE:
        _PROGRAM_CACHE[key] = _build_program(pos, nch, m)
    nc = _PROGRAM_CACHE[key]

    from concourse.bass_utils import run_bass_kernel_spmd

    res = run_bass_kernel_spmd(
        nc,
        in_maps,
        core_ids=list(range(NCORES)),
        trace=bool(os.environ.get("BASS_TRACE")),
    )
    LAST_RESULTS = res

    out = np.zeros((B, E), np.float64)
    for c in range(NCORES):
        out += np.asarray(res.results[c]["outT"]).astype(np.float64)
    return out.astype(np.float32)


# revision 65
# speedup vs baseline: 1.0110x; 1.0110x over previous
"""Paged KV-cache decode attention with ALiBi (Baichuan-style), fused
QKV + attention + output projection, tensor-parallel over heads across
8 Trainium2 NeuronCores.

v6: fp16 everywhere + ALiBi window truncation with balanced head
permutation.

ALiBi slopes decay geometrically, so for most heads only the last
`win_h = ceil(40/slope_h)` positions can contribute: everything earlier
has additive bias < -40, i.e. softmax weight < e^-34 ~ 1e-15 of the
winner. We therefore (a) rank all 40 heads by window size, (b) give
each core one head from each octile (balanced shards: every core gets
the same per-slot chunk capacities, so one shared SPMD program works),
and (c) host-pack ONLY the needed trailing chunks of K^T/V per
(slot, seq). This cuts K/V HBM traffic AND the attention matmul count
by ~45% on every core.

Layout strategy (per core, 5 head-slots):
  - qT/kT computed as [640, 4] (head-dim on partitions) so scores matmuls
    need no transposes and the K-cache new-token scatter is a same-partition
    SBUF copy.
  - v computed as [4, 640] (natural) so the V new-token scatter is a tiny
    SBUF->SBUF DMA row write (scalar/HWDGE queue, grouped per head).
  - K packed host-side as [128(d), sum_chunks*128] (K^T), V as
    [128(t%128), sum_chunks, 128(d)]; ONE DMA per head-slot, single-use
    exact-size SBUF tiles (no pool-reuse waits in the DMA stream).
  - One explicitly-ordered bulk DMA stream on the gpsimd queue:
    weights first (the last v-weight chunk gates all attention), then
    K/V slots, then o_proj weights (consumed at DMA pace at the end).
  - Per-head attention emits all 4 seqs' score matmuls before the 4 AV
    chains so the exp round-trips hide behind other seqs' scores.
  - softmax without max-subtraction; masking via host-precomputed
    additive fp32 bias (-1e30).
  - o_proj in natural orientation (lhsT = tiny attn columns, ow rides
    the 512-wide moving side); host sums the 8 partial outputs.
"""

import math
import os
import sys
from contextlib import ExitStack

import numpy as np

sys.path.insert(0, "/opt/trn_rl_repo")

B = 4
E = 5120
H = 40
D = 128
BS = 16
NB = 512
MB = 128
S = MB * BS  # 2048
NCORES = 8
HPC = H // NCORES   # 5 head-slots per core
EPC = HPC * D       # 640

NEG = -1.0e30
GK = 10             # E-chunks (of 128) per qkv weight DMA group
TCUT = 26.0         # alibi bias cutoff: positions with bias < -TCUT dropped
                    # (dropped softmax weight <= ~e^-14 relative: negligible)


def _alibi_slopes(num_heads):
    cp2 = 2 ** int(math.floor(math.log2(num_heads)))
    base = 2.0 ** (-(2.0 ** (-(math.log2(cp2) - 3))))
    slopes = base ** np.arange(1, cp2 + 1, dtype=np.float64)
    if cp2 != num_heads:
        extra_base = 2.0 ** (-(2.0 ** (-(math.log2(2 * cp2) - 3))))
        n_rem = min(cp2, num_heads - cp2)
        extra = extra_base ** np.arange(1, 1 + 2 * n_rem, 2, dtype=np.float64)
        slopes = np.concatenate([slopes, extra])
    return slopes.astype(np.float32)


def _head_partition(pos, nch):
    """Rank heads by alibi window, assign core c slot s <- rank[s*8+c].
    Returns (order, m) where m[s][b] = kept trailing chunks for slot s."""
    win = np.ceil(TCUT / _alibi_slopes(H).astype(np.float64)).astype(np.int64)
    order = np.argsort(win, kind="stable")
    m = []
    for s in range(HPC):
        wmax = int(win[order[s * NCORES:(s + 1) * NCORES]].max())
        m.append(tuple(nch[b] - max(0, (pos[b] - wmax) // 128) for b in range(B)))
    return order, tuple(m)


_PROGRAM_CACHE = {}
LAST_RESULTS = None  # BassKernelResults of the most recent run (for test.py)


def _build_program(pos, nch, m):
    """Build the SPMD Bass program. pos/nch/m are baked statically (same
    for all cores; per-core data varies only via inputs)."""
    import concourse.bacc as bacc
    import concourse.bass as bass
    import concourse.tile as tile
    from concourse import mybir

    f32 = mybir.dt.float32
    f16 = mybir.dt.float16
    nc = bacc.Bacc()

    scnt = [sum(m[s]) for s in range(HPC)]   # chunks per slot
    soff = [0]
    for s in range(HPC):
        soff.append(soff[-1] + scnt[s])
    KCH = soff[-1]
    # chunk offset of (s, b) within slot s's tile
    moff = [[sum(m[s][:b]) for b in range(B)] for s in range(HPC)]
    c0 = [[nch[b] - m[s][b] for b in range(B)] for s in range(HPC)]

    hT = nc.declare_dram_parameter("hT", [128, 40 * B], f16, isOutput=False)
    qkvw = nc.declare_dram_parameter("qkvw", [3, 128, 40, EPC], f16, isOutput=False)
    ow = nc.declare_dram_parameter("ow", [128, HPC * E], f16, isOutput=False)
    kt = nc.declare_dram_parameter("kt", [128, KCH * 128], f16, isOutput=False)
    vt = nc.declare_dram_parameter("vt", [128, KCH, D], f16, isOutput=False)
    bias = nc.declare_dram_parameter("bias", [128, B * HPC * 16], f32, isOutput=False)
    outT = nc.declare_dram_parameter("outT", [B, E], f32, isOutput=True)

    NG = 40 // GK  # weight DMA groups per tensor

    with tile.TileContext(nc) as tc, ExitStack() as ctx:
        consts = ctx.enter_context(tc.tile_pool(name="consts", bufs=1))
        wpool = ctx.enter_context(tc.tile_pool(name="wpool", bufs=4))
        tmp = ctx.enter_context(tc.tile_pool(name="tmp", bufs=4))
        psum = ctx.enter_context(tc.tile_pool(name="psum", bufs=8, space="PSUM"))

        hT_sb = consts.tile([128, 40 * B], f16)          # (E%128, (Echunk, b))
        bias_sb = consts.tile([128, B * HPC * 16], f32)  # (t%128, (b, s, chunk))
        ow_sb = consts.tile([128, HPC * E], f16)
        qT_sb = consts.tile([128, HPC * B], f16)   # col = s*B + b ; partition = d
        kT_sb = consts.tile([128, HPC * B], f16)
        v_sb = consts.tile([B, EPC], f16)          # natural v rows
        colsum_sb = consts.tile([128, HPC * B], f32)
        aoT_sb = consts.tile([128, HPC * B], f32)  # unnormalized attn@V ^T
        out_sb = consts.tile([B, E], f32)          # natural o_proj output

        ones_col = consts.tile([128, 1], f32)
        nc.vector.memset(ones_col[:], 1.0)
        ones_row = consts.tile([1, 128], f32)
        nc.vector.memset(ones_row[:], 1.0)

        # per-slot single-use exact-size K/V tiles (no pool-reuse waits)
        Kts = [consts.tile([128, scnt[s] * 128], f16, name=f"K{s}") for s in range(HPC)]
        Vts = [consts.tile([128, scnt[s], D], f16, name=f"V{s}") for s in range(HPC)]

        # ---- the bulk DMA stream: ONE queue (gpsimd/SWDGE), explicitly
        # ordered. All weight groups land before the K/V slots (the last
        # v-weight chunk gates the whole attention phase); ow last so the
        # o_proj tail is DMA-paced.
        wq, wk, wv = [], [], []

        def qkv_group(w, lst):
            t = wpool.tile([128, GK, EPC], f16, tag="w", name=f"w{w}_{len(lst)}")
            nc.gpsimd.dma_start(
                out=t[:], in_=qkvw[w, :, len(lst) * GK:(len(lst) + 1) * GK, :]
            )
            lst.append(t)

        nc.gpsimd.dma_start(out=hT_sb[:], in_=hT[:])
        nc.gpsimd.dma_start(out=bias_sb[:], in_=bias[:])
        for g in range(NG):
            qkv_group(0, wq)
        nc.gpsimd.dma_start(out=Kts[HPC - 1][:],
                            in_=kt[:, soff[HPC - 1] * 128: soff[HPC] * 128])
        for g in range(NG):
            qkv_group(1, wk)
        # K/V slots stream LARGEST-first (slot 4 = full window, 44 chunks;
        # slot 0 = 5 chunks) and the attention loop runs in the same order:
        # the last-arriving tile then gates only ~2us of work instead of
        # the biggest slot's ~8us, shrinking the post-DMA attention tail.
        for s in range(HPC - 2, -1, -1):
            nc.gpsimd.dma_start(out=Kts[s][:], in_=kt[:, soff[s] * 128: soff[s + 1] * 128])
        for g in range(NG):
            qkv_group(2, wv)
        for s in range(HPC - 1, -1, -1):
            nc.gpsimd.dma_start(out=Vts[s][:], in_=vt[:, soff[s]: soff[s + 1], :])
        # ow in jg-major chunks: o_proj group jg only needs chunk jg//2,
        # so the projection pipelines at DMA pace behind the stream tail.
        for oc in range(5):
            w = HPC * E // 5
            nc.gpsimd.dma_start(out=ow_sb[:, oc * w:(oc + 1) * w], in_=ow[:, oc * w:(oc + 1) * w])

        # ---- fused QKV projection ----
        # q,k transposed orientation: psum[oc] [128, B] accumulated over 40
        # E-chunks; lhsT = W chunk [128(E), 128(outcol)], rhs = hT chunk [128(E), B].
        for w, lst in ((0, wq), (1, wk)):  # 0=q (pre-scaled on host), 1=k
            dst = qT_sb if w == 0 else kT_sb
            ps = [psum.tile([128, B], f32, tag="ps", name=f"ps_qk{w}_{i}") for i in range(HPC)]
            for g in range(NG):
                wt = lst[g]
                for oc in range(HPC):
                    for kl in range(GK):
                        kc = g * GK + kl
                        nc.tensor.matmul(
                            ps[oc][:],
                            lhsT=wt[:, kl, oc * 128:(oc + 1) * 128],
                            rhs=hT_sb[:, kc * B:(kc + 1) * B],
                            start=(kc == 0),
                            stop=(kc == 39),
                        )
            for oc in range(HPC):
                nc.scalar.copy(dst[:, oc * B:(oc + 1) * B], ps[oc][:])

        # v natural orientation: psum [B, 640] (two banks: 512 + 128),
        # lhsT = hT chunk [128(E), B], rhs = Wv chunk [128(E), 640].
        v_ps0 = psum.tile([B, 512], f32, tag="ps")
        v_ps1 = psum.tile([B, EPC - 512], f32, tag="ps")
        for g in range(NG):
            wt = wv[g]
            for kl in range(GK):
                kc = g * GK + kl
                nc.tensor.matmul(
                    v_ps0[:],
                    lhsT=hT_sb[:, kc * B:(kc + 1) * B],
                    rhs=wt[:, kl, :512],
                    start=(kc == 0),
                    stop=(kc == 39),
                )
                nc.tensor.matmul(
                    v_ps1[:],
                    lhsT=hT_sb[:, kc * B:(kc + 1) * B],
                    rhs=wt[:, kl, 512:],
                    start=(kc == 0),
                    stop=(kc == 39),
                )
        nc.scalar.copy(v_sb[:, :512], v_ps0[:])
        nc.scalar.copy(v_sb[:, 512:], v_ps1[:])

        # ---- attention, head-slot-major, LARGEST slot first (matching the
        # K/V stream order); per head: scatters, then all 4 seqs' scores,
        # then adds/exps, then the 4 AV chains, so the DVE/ACT round-trips
        # hide behind other seqs' score matmuls.
        for s in range(HPC - 1, -1, -1):
            Kt = Kts[s]
            Vt = Vts[s]
            lpos = [moff[s][b] * 128 + (pos[b] // 128 - c0[s][b]) * 128 + pos[b] % 128
                    for b in range(B)]
            # V new-token scatter rows (cross-partition -> DMA). On the sync
            # queue, which is otherwise idle until the final store: the
            # issue waits (v_sb + Vt arrival) can't block exps or adds.
            for b in range(B):
                p = pos[b]
                nc.sync.dma_start(
                    out=Vt[p % 128: p % 128 + 1, moff[s][b] + p // 128 - c0[s][b], :],
                    in_=v_sb[b:b + 1, s * D:(s + 1) * D],
                )
            # K new-token scatter columns (same partitions: DVE copy)
            for b in range(B):
                nc.vector.tensor_copy(
                    Kt[:, lpos[b]: lpos[b] + 1], kT_sb[:, (s * B + b):(s * B + b) + 1]
                )
            sc_ps = [psum.tile([128, 16], f32, tag="ps", name=f"sc_{s}_{b}") for b in range(B)]
            for b in range(B):
                for c in range(m[s][b]):
                    nc.tensor.matmul(
                        sc_ps[b][:, c:c + 1],
                        lhsT=Kt[:, (moff[s][b] + c) * 128:(moff[s][b] + c + 1) * 128],
                        rhs=qT_sb[:, s * B + b: s * B + b + 1],
                        start=True,
                        stop=True,
                    )
            attn = []
            for b in range(B):
                n = m[s][b]
                col = s * B + b
                s_sb = tmp.tile([128, 16], f32, tag="s", name=f"s_{s}_{b}")
                nc.vector.tensor_add(
                    s_sb[:, :n],
                    sc_ps[b][:, :n],
                    bias_sb[:, (b * HPC + s) * 16:(b * HPC + s) * 16 + n],
                )
                attn_sb = tmp.tile([128, 16], f16, tag="attn", name=f"at_{s}_{b}")
                nc.scalar.activation(
                    attn_sb[:, :n],
                    s_sb[:, :n],
                    func=mybir.ActivationFunctionType.Exp,
                    accum_out=colsum_sb[:, col:col + 1],
                )
                attn.append(attn_sb)
            for b in range(B):
                n = m[s][b]
                col = s * B + b
                ao_ps = psum.tile([128, 1], f32, tag="ps", name=f"ao_{s}_{b}")
                for c in range(n):
                    nc.tensor.matmul(
                        ao_ps[:],
                        lhsT=Vt[:, moff[s][b] + c, :],
                        rhs=attn[b][:, c:c + 1],
                        start=(c == 0),
                        stop=(c == n - 1),
                    )
                nc.scalar.copy(aoT_sb[:, col:col + 1], ao_ps[:])

        # ---- softmax normalization (batched over all 20 (s,b)) ----
        sums_ps = psum.tile([1, HPC * B], f32, tag="ps")
        nc.tensor.matmul(
            sums_ps[:], lhsT=ones_col[:], rhs=colsum_sb[:], start=True, stop=True
        )
        recip_sb = tmp.tile([1, HPC * B], f32, tag="recip")
        nc.vector.reciprocal(recip_sb[:], sums_ps[:])
        rb_ps = psum.tile([128, HPC * B], f32, tag="ps")
        nc.tensor.matmul(
            rb_ps[:], lhsT=ones_row[:], rhs=recip_sb[:], start=True, stop=True
        )
        recip_b = tmp.tile([128, HPC * B], f32, tag="recipb")
        nc.vector.tensor_copy(recip_b[:], rb_ps[:])
        attn_nT = consts.tile([128, HPC * B], f16)
        nc.vector.tensor_mul(attn_nT[:], aoT_sb[:], recip_b[:])

        # ---- output projection (natural): out[b, j] ----
        # lhsT = attn_nT slice [128(hd), B] (4-col weight load, ~free);
        # rhs = ow chunk [128(hd%128), 512] moving at 1 col/cycle.
        for jg in range(E // 512):
            ops = psum.tile([B, 512], f32, tag="ps", name=f"ps_o{jg}")
            for hh in range(HPC):
                j0 = jg * (HPC * 512) + hh * 512
                nc.tensor.matmul(
                    ops[:],
                    lhsT=attn_nT[:, hh * B:(hh + 1) * B],
                    rhs=ow_sb[:, j0: j0 + 512],
                    start=(hh == 0),
                    stop=(hh == HPC - 1),
                )
            # alternate evacuation engines so the copy chain pipelines
            if jg % 2 == 0:
                nc.scalar.copy(out_sb[:, jg * 512:(jg + 1) * 512], ops[:])
            else:
                nc.vector.tensor_copy(out_sb[:, jg * 512:(jg + 1) * 512], ops[:])

        nc.sync.dma_start(out=outT[:, :E // 2], in_=out_sb[:, :E // 2])
        nc.sync.dma_start(out=outT[:, E // 2:], in_=out_sb[:, E // 2:])

    nc.compile()  # Bacc finalize: splits multi-waits (matmul 1-wait limit)
    return nc


def _prepare_core_inputs(core, hidden16, qkv16, o16, k16, v16, bt, sl, pos, nch,
                         order, m):
    """Per-core staged arrays with the window-permuted head assignment."""
    heads = [int(order[s * NCORES + core]) for s in range(HPC)]

    # partition-major: qkvw[w, p, kc, c] = W[w, kc*128 + p, head cols c]
    qkvw = np.ascontiguousarray(
        qkv16.reshape(3, E, H, D)[:, :, heads, :]
        .reshape(3, 40, 128, EPC).transpose(0, 2, 1, 3)
    )

    scnt = [sum(m[s]) for s in range(HPC)]
    KCH = sum(scnt)
    moff = [[sum(m[s][:b]) for b in range(B)] for s in range(HPC)]
    soff = [0]
    for s in range(HPC):
        soff.append(soff[-1] + scnt[s])
    c0 = [[nch[b] - m[s][b] for b in range(B)] for s in range(HPC)]

    kg = k16[:, heads]  # [NB, HPC, BS, D]
    vg = v16[:, heads]
    kt = np.zeros((D, KCH * 128), np.float16)
    vt = np.zeros((128, KCH, D), np.float16)
    for b in range(B):
        sd = nch[b] * 128
        blocks = bt[b][: sd // BS]
        kk = kg[blocks].transpose(1, 0, 2, 3).reshape(HPC, sd, D)
        vv = vg[blocks].transpose(1, 0, 2, 3).reshape(HPC, sd, D)
        for s in range(HPC):
            base = soff[s] + moff[s][b]
            n = m[s][b]
            ksl = kk[s, c0[s][b] * 128: sd]              # [n*128, D]
            kt[:, base * 128: (base + n) * 128] = ksl.T
            vt[:, base: base + n, :] = vv[s, c0[s][b] * 128: sd].reshape(
                n, 128, D).transpose(1, 0, 2)

    slopes = _alibi_slopes(H)[heads]
    t_in = np.arange(128)[:, None]
    biasa = np.full((128, B, HPC, 16), NEG, np.float32)
    for b in range(B):
        for s in range(HPC):
            n = m[s][b]
            tg = ((c0[s][b] + np.arange(n))[None, :] * 128 + t_in).astype(np.float32)
            val = slopes[s] * (tg - np.float32(pos[b]))
            val[tg >= sl[b]] = NEG
            biasa[:, b, s, :n] = val

    hTf = np.ascontiguousarray(
        hidden16.T.reshape(40, 128, B).transpose(1, 0, 2).reshape(128, 40 * B)
    )

    # ow pre-transposed, jg-major: owr[p, jg*HPC*512 + s*512 + j'] =
    # o_proj_weight[heads[s]*128 + p, jg*512 + j']
    owr = np.ascontiguousarray(
        o16.reshape(H, D, E)[heads].reshape(HPC, 128, E // 512, 512)
        .transpose(1, 2, 0, 3).reshape(128, HPC * E)
    )

    return dict(
        hT=hTf,
        qkvw=qkvw,
        ow=owr,
        kt=kt,
        vt=vt,
        bias=np.ascontiguousarray(biasa.reshape(128, B * HPC * 16)),
    )


def kernel(**inputs):
    global LAST_RESULTS
    hidden = np.asarray(inputs["hidden_states"], np.float32)
    qkv_w = np.asarray(inputs["qkv_weight"], np.float32)
    o_w = np.asarray(inputs["o_proj_weight"], np.float32)
    k_cache = np.asarray(inputs["k_cache"], np.float32)
    v_cache = np.asarray(inputs["v_cache"], np.float32)
    bt = np.asarray(inputs["block_tables"]).astype(np.int64)
    sl = np.asarray(inputs["sequence_lengths"]).astype(np.int64)

    pos = tuple(int(x) - 1 for x in sl)
    nch = tuple(int(math.ceil(int(x) / 128)) for x in sl)
    order, m = _head_partition(pos, nch)

    # cast once to fp16 (q pre-scaled by 1/sqrt(D) before the cast)
    hidden16 = hidden.astype(np.float16)
    qkv16 = qkv_w.copy()
    qkv16[0] *= np.float32(D ** -0.5)
    qkv16 = qkv16.astype(np.float16)
    o16 = o_w.astype(np.float16)
    k16 = k_cache.astype(np.float16)
    v16 = v_cache.astype(np.float16)

    in_maps = [
        _prepare_core_inputs(c, hidden16, qkv16, o16, k16, v16, bt, sl, pos, nch,
                             order, m)
        for c in range(NCORES)
    ]

    key = (pos, nch, m)
    if key not in _PROGRAM_CACHE:
        _PROGRAM_CACHE[key] = _build_program(pos, nch, m)
    nc = _PROGRAM_CACHE[key]

    from concourse.bass_utils import run_bass_kernel_spmd

    res = run_bass_kernel_spmd(
        nc,
        in_maps,
        core_ids=list(range(NCORES)),
        trace=bool(os.environ.get("BASS_TRACE")),
    )
    LAST_RESULTS = res

    out = np.zeros((B, E), np.float64)
    for c in range(NCORES):
        out += np.asarray(res.results[c]["outT"]).astype(np.float64)
    return out.astype(np.float32)
